# revision 61
# baseline (speedup 1.0000x reference)
"""Trainium2 Bass kernel for nn_AttnResLayer (sparse_attention).

Computes, for V [N=12, B=4, T=2048, D=1024] fp32:
  K = rmsnorm(V) * norm_weight
  logits[n,b,t] = dot(w_l, K[n,b,t,:])
  alpha = softmax(logits, axis=n)
  out[b,t,d] = sum_n alpha[n,b,t] * V[n,b,t,d]

Sharding: T split across 8 cores (256 tokens/core per b); w_l/norm_weight
replicated (folded into one weight row host-side). No collectives.

fp16 transport: V is cast to fp16 on the host (and the output returned as
fp16, upcast on the host), halving HBM traffic vs fp32 — the op only needs
rel_err < 2e-2 and the fp16 pipeline measures ~1.1e-2 end to end. All
reductions accumulate in fp32 on-chip.

Three-stage software pipeline over 128-token chunks: in period i the
engines interleave R(i) (loads + squares/dots), S(i-1) (softmax smalls,
diag builds, dense MAC burst) and D(i-2) (PSUM drain). Queue orders avoid
head-of-line blocking: every cross-engine-dependent small op has
independent work ahead of it in its queue, and the DVE sums of the
Pool-made products sit at the points where the Pool TTs actually
complete. The MAC burst is gated only on the diags (the two DVE-built
ones consumed first); two late pacing matmuls on the Pool products plus
the slice-paced warms of chunk 0 keep the PE p-state from long cold
idles. The weight row and fp16 identity are DMAed from the host (the
fp16 bus has ~50% headroom, so on-chip generation only cost time).

Per-chunk engine balance (12 slices of [128, 1024] fp16):
  ACT : 9 squares of slices 3..11 (7 with fused fp32 accum at 1225ns;
        2 without the 187ns accum-read, summed on DVE @4x) + ln/exp/
        softmax smalls + the whole PSUM drain (Copy, ~1223ns)
  DVE : 9 dot TT(v,wb)@2x + 14 tensor_scalar-sums@4x + square TT+sum
        pairs for the early slices 0,1,2 (shortens the ss relay into the
        next period's softmax) + softmax smalls + diags 10,11
  Pool: 3 dot TT multiplies (slices 2,5,8) + 10 diag TTs
  PE  : dense 24-matmul MAC burst + 3 pacing matmuls on Pool products
All activations pinned to the table set holding ln+exp+square+copy.
All output stores issue after the last load so the fp16 store traffic
covers the final chunk's compute tail. Timeline-sim: 143.7us vs 154.9us
for the fp32 baseline; engine-bound (ACT ~13.5us/chunk) above the fp16
DMA roofline of ~76us.
"""

import numpy as np
from contextlib import ExitStack

import concourse.bass as bass
import concourse.bacc as bacc
import concourse.tile as tile
from concourse import mybir
from concourse.bass_utils import run_bass_kernel_spmd


def _pinned_tables(arch, _orig=bacc.get_activation_tables):
    tables = _orig(arch)
    keep = "natural_log_exp_and_others"
    return {k: (v if k == keep else set()) for k, v in tables.items()}


N, B, T, D = 12, 4, 2048, 1024
NCORES = 8
TSH = T // NCORES
P = 128
NCHUNK = TSH // P
NCK = B * NCHUNK
EPS = 1e-6
FP32 = mybir.dt.float32
FP16 = mybir.dt.float16
AF = mybir.ActivationFunctionType
ALU = mybir.AluOpType

POOL_DOT = (2, 5, 8)      # dot-product TT multiplies on Pool
DVE_SQ = (0, 1, 2)        # squares on DVE (TT+sum); rest on ACT
N_POOL_DIAG = 10          # diags 0..9 on Pool; 10,11 on DVE
MAC_ORDER = (10, 11, 0, 1, 2, 3, 4, 5, 6, 7, 8, 9)
MAC_AFTER_WARM = 7        # dense MAC burst after this many warms


def _build_nc() -> bacc.Bacc:
    nc = bacc.Bacc("TRN2", target_bir_lowering=False, debug=False,
                   num_devices=NCORES)
    v_in = nc.dram_tensor("v", [N, B, TSH, D], FP16, kind="ExternalInput").ap()
    wb_in = nc.dram_tensor("wbt", [P, D], FP16, kind="ExternalInput").ap()
    ones_in = nc.dram_tensor("idm", [P, P], FP16, kind="ExternalInput").ap()
    out_d = nc.dram_tensor("out", [B, TSH, D], FP16, kind="ExternalOutput").ap()

    orig_tables = bacc.get_activation_tables
    bacc.get_activation_tables = _pinned_tables
    try:
        _build_body(nc, v_in, wb_in, ones_in, out_d)
    finally:
        bacc.get_activation_tables = orig_tables
    return nc


def _build_body(nc, v_in, wb_in, ones_in, out_d):
    with tile.TileContext(nc) as tc, ExitStack() as ctx:
        const_pool = ctx.enter_context(tc.tile_pool(name="const", bufs=1))
        v_pool = ctx.enter_context(tc.tile_pool(name="vp", bufs=3))
        scr_pool = ctx.enter_context(tc.tile_pool(name="scr", bufs=1))
        scrp_pool = ctx.enter_context(tc.tile_pool(name="scrp", bufs=3))
        scrq_pool = ctx.enter_context(tc.tile_pool(name="scrq", bufs=3))
        small_pool = ctx.enter_context(tc.tile_pool(name="small", bufs=3))
        diag_pool = ctx.enter_context(tc.tile_pool(name="diag", bufs=24))
        psum_pool = ctx.enter_context(
            tc.tile_pool(name="accp", bufs=2, space="PSUM"))
        warm_pool = ctx.enter_context(
            tc.tile_pool(name="warmp", bufs=1, space="PSUM"))
        out_pool = ctx.enter_context(tc.tile_pool(name="outp", bufs=8))

        eps_t = const_pool.tile([P, 1], FP32, name="eps_t")
        nc.vector.memset(eps_t[:], EPS)
        id16 = const_pool.tile([P, P], FP16, name="id16")
        nc.scalar.dma_start(id16[:], ones_in[:])
        wb_t = const_pool.tile([P, D], FP16, name="wb_t")
        nc.scalar.dma_start(wb_t[:], wb_in[:])
        scr_act = scr_pool.tile([P, D], FP16, name="scr_act")
        scr_dve = scr_pool.tile([P, D], FP16, name="scr_dve")

        stores = []
        pend = None   # chunk awaiting S (reductions done)
        macd = None   # chunk awaiting D (MAC done, drain pending)

        def dve_sq(q, vts, ss):
            nc.vector.tensor_tensor(out=scr_dve[:], in0=vts[q],
                                    in1=vts[q], op=ALU.mult)
            nc.vector.tensor_scalar(
                out=scr_dve[:], in0=scr_dve[:], scalar1=1.0, scalar2=0.0,
                op0=ALU.mult, op1=ALU.add, accum_out=ss[:, q:q + 1])

        def dve_dot(q, vts, dot):
            nc.vector.tensor_tensor(out=scr_dve[:], in0=vts[q],
                                    in1=wb_t[:], op=ALU.mult)
            nc.vector.tensor_scalar(
                out=scr_dve[:], in0=scr_dve[:], scalar1=1.0, scalar2=0.0,
                op0=ALU.mult, op1=ALU.add, accum_out=dot[:, q:q + 1])

        def drain(st, last_piece=None):
            with tc.high_priority(offset=-100):
                nc.scalar.activation(st["out"][:], st["acc"][:], AF.Copy)
            stores.append((out_d[st["b"], st["t0"]:st["t0"] + P, :],
                           st["out"][:]))

        for ci in range(NCK):
            b, c = divmod(ci, NCHUNK)
            t0 = c * P
            # ---- R(ci): loads ----
            vslices = []
            for q in range(N):
                vt = v_pool.tile([P, D], FP16, name=f"vs{q}", tag=f"vs{q}")
                nc.sync.dma_start(vt[:], v_in[q, b, t0:t0 + P, :])
                vslices.append(vt)
            vts = [vslices[q][:] for q in range(N)]

            ss = small_pool.tile([P, N], FP32, name="ss", tag="ss")
            dot = small_pool.tile([P, N], FP32, name="dot", tag="dot")
            pool_prods = {}

            def pool_tt(q):
                sp = scrp_pool.tile([P, D], FP16, name="scrp", tag="scrp")
                nc.gpsimd.tensor_tensor(out=sp[:], in0=vts[q], in1=wb_t[:],
                                        op=ALU.mult)
                pool_prods[q] = sp

            def pool_tsum(q):
                nc.vector.tensor_scalar(
                    out=pool_prods[q][:], in0=pool_prods[q][:],
                    scalar1=1.0, scalar2=0.0, op0=ALU.mult, op1=ALU.add,
                    accum_out=dot[:, q:q + 1])

            # Pool: first dot TT (slice 2) fills Pool before diags(ci-1)
            pool_tt(POOL_DOT[0])

            # ACT head: Ln/Exp(ci-1) — ss(ci-1) complete, never blocks
            if pend is not None:
                u = small_pool.tile([P, N], FP32, name="u", tag="u")
                nc.scalar.activation(u[:], pend["ss"][:], AF.Ln,
                                     bias=eps_t[:, 0:1], scale=1.0 / D)
                rms = small_pool.tile([P, N], FP32, name="rms", tag="rms")
                nc.scalar.activation(rms[:], u[:], AF.Exp, scale=-0.5)
            # DVE head: logits smalls
            if pend is not None:
                logits = small_pool.tile([P, N], FP32, name="lg", tag="lg")
                nc.vector.tensor_mul(logits[:], pend["dot"][:], rms[:])
                negmax = small_pool.tile([P, 1], FP32, name="nm", tag="nm")
                nc.vector.tensor_reduce(negmax[:], logits[:],
                                        axis=mybir.AxisListType.X,
                                        op=ALU.max, negate=True)
            # ACT: first square, then aexp (no accum; sum on DVE)
            act_sq = [q for q in range(N) if q not in DVE_SQ]
            nc.scalar.activation(scr_act[:], vts[act_sq[0]], AF.Square,
                                 accum_out=ss[:, act_sq[0]:act_sq[0] + 1])
            if pend is not None:
                aexp = small_pool.tile([P, N], FP32, name="ax", tag="ax")
                nc.scalar.activation(aexp[:], logits[:], AF.Exp,
                                     bias=negmax[:, 0:1])
            # DVE: dot(0) fills the gap, then finish the alpha chain
            dve_dots = [q for q in range(N) if q not in POOL_DOT]
            dve_dot(dve_dots[0], vts, dot)
            if pend is not None:
                sumexp = small_pool.tile([P, 1], FP32, name="se", tag="se")
                nc.vector.tensor_scalar(
                    out=aexp[:], in0=aexp[:], scalar1=1.0, scalar2=0.0,
                    op0=ALU.mult, op1=ALU.add, accum_out=sumexp[:])
                recip = small_pool.tile([P, 1], FP32, name="rc", tag="rc")
                nc.vector.reciprocal(recip[:], sumexp[:])
                anorm = small_pool.tile([P, N], FP32, name="an", tag="an")
                nc.vector.tensor_scalar(out=anorm[:], in0=aexp[:],
                                        scalar1=recip[:, 0:1], scalar2=1.0,
                                        op0=ALU.mult, op1=ALU.mult)
                dgs = {}
                for n in range(N_POOL_DIAG, N):
                    dg = diag_pool.tile([P, P], FP16, name="dg", tag="dg")
                    nc.vector.tensor_scalar(out=dg[:], in0=id16[:],
                                            scalar1=anorm[:, n:n + 1],
                                            scalar2=1.0,
                                            op0=ALU.mult, op1=ALU.mult)
                    dgs[n] = dg
                for n in range(N_POOL_DIAG):
                    dg = diag_pool.tile([P, P], FP16, name="dg", tag="dg")
                    nc.gpsimd.tensor_tensor(
                        out=dg[:], in0=id16[:],
                        in1=anorm[:, n:n + 1].broadcast_to([P, P]),
                        op=ALU.mult)
                    dgs[n] = dg
                pacc = psum_pool.tile([P, D], FP32, name="acc", tag="acc")
                pout = out_pool.tile([P, D], FP16, name="out_sb", tag="ot")

            # ACT: rest of squares (last two without the 187ns accum-read;
            # their sums run on DVE @4x), then drain(ci-2) at the queue end
            nc.scalar.activation(scr_act[:], vts[act_sq[1]], AF.Square,
                                 accum_out=ss[:, act_sq[1]:act_sq[1] + 1])
            sq_scr = {}
            for q in act_sq[2:]:
                if q in act_sq[-2:]:
                    sq = scrq_pool.tile([P, D], FP16, name="scrq", tag="scrq")
                    nc.scalar.activation(sq[:], vts[q], AF.Square)
                    sq_scr[q] = sq
                else:
                    nc.scalar.activation(scr_act[:], vts[q], AF.Square,
                                         accum_out=ss[:, q:q + 1])
            if macd is not None:
                drain(macd)
                macd = None

            # Pool: remaining dot TTs
            for q in POOL_DOT[1:]:
                pool_tt(q)

            # PE: dense MAC burst paced only by diag arrival; two late
            # pacing matmuls on the Pool products bridge the period
            # boundary so the p-state never sees a long cold idle.
            warm_ps = warm_pool.tile([P, 512], FP32, name="warm_ps", tag="wp")
            if ci == 0:
                for k in range(N):
                    nc.tensor.matmul(warm_ps[:], id16[:], vts[k][:, 0:512],
                                     start=True, stop=True)
            if pend is not None:
                for k, n in enumerate(MAC_ORDER):
                    for h in range(2):
                        nc.tensor.matmul(pacc[:, h * 512:(h + 1) * 512],
                                         dgs[n][:],
                                         pend["vts"][n][:,
                                                        h * 512:(h + 1) * 512],
                                         start=(k == 0), stop=(k == N - 1))

            # DVE: remaining dots + squares; pool sums + pacing warms
            for q in dve_dots[1:]:
                dve_dot(q, vts, dot)
            for q in DVE_SQ:
                dve_sq(q, vts, ss)
            for q in POOL_DOT[:-1]:
                pool_tsum(q)
                nc.tensor.matmul(warm_ps[:], id16[:],
                                 pool_prods[q][:, 0:512],
                                 start=True, stop=True)
            for q, sq in sq_scr.items():
                nc.vector.tensor_scalar(
                    out=sq[:], in0=sq[:], scalar1=1.0, scalar2=0.0,
                    op0=ALU.mult, op1=ALU.add, accum_out=ss[:, q:q + 1])
            pool_tsum(POOL_DOT[-1])
            nc.tensor.matmul(warm_ps[:], id16[:],
                             pool_prods[POOL_DOT[-1]][:, 0:512],
                             start=True, stop=True)

            if pend is not None:
                macd = {"acc": pacc, "out": pout,
                        "b": pend["b"], "t0": pend["t0"]}
            pend = {"ss": ss, "dot": dot, "vts": vts, "b": b, "t0": t0}

        # ---- tail: S(NCK-1) smalls first, then drain(NCK-2) ----
        u = small_pool.tile([P, N], FP32, name="u", tag="u")
        nc.scalar.activation(u[:], pend["ss"][:], AF.Ln,
                             bias=eps_t[:, 0:1], scale=1.0 / D)
        rms = small_pool.tile([P, N], FP32, name="rms", tag="rms")
        nc.scalar.activation(rms[:], u[:], AF.Exp, scale=-0.5)
        logits = small_pool.tile([P, N], FP32, name="lg", tag="lg")
        nc.vector.tensor_mul(logits[:], pend["dot"][:], rms[:])
        negmax = small_pool.tile([P, 1], FP32, name="nm", tag="nm")
        nc.vector.tensor_reduce(negmax[:], logits[:],
                                axis=mybir.AxisListType.X,
                                op=ALU.max, negate=True)
        aexp = small_pool.tile([P, N], FP32, name="ax", tag="ax")
        sumexp = small_pool.tile([P, 1], FP32, name="se", tag="se")
        nc.scalar.activation(aexp[:], logits[:], AF.Exp,
                             bias=negmax[:, 0:1], accum_out=sumexp[:])
        recip = small_pool.tile([P, 1], FP32, name="rc", tag="rc")
        nc.vector.reciprocal(recip[:], sumexp[:])
        anorm = small_pool.tile([P, N], FP32, name="an", tag="an")
        nc.vector.tensor_scalar(out=anorm[:], in0=aexp[:],
                                scalar1=recip[:, 0:1], scalar2=1.0,
                                op0=ALU.mult, op1=ALU.mult)
        if macd is not None:
            drain(macd)
        warm_ps = warm_pool.tile([P, 512], FP32, name="warm_ps", tag="wp")
        for _ in range(14):
            nc.tensor.matmul(warm_ps[:], id16[:],
                             pend["vts"][N - 1][:, 0:512],
                             start=True, stop=True)
        dgs = {}
        for n in range(N):
            dg = diag_pool.tile([P, P], FP16, name="dg", tag="dg")
            if n % 2 == 0:
                nc.gpsimd.tensor_tensor(
                    out=dg[:], in0=id16[:],
                    in1=anorm[:, n:n + 1].broadcast_to([P, P]), op=ALU.mult)
            else:
                nc.vector.tensor_scalar(out=dg[:], in0=id16[:],
                                        scalar1=anorm[:, n:n + 1],
                                        scalar2=1.0,
                                        op0=ALU.mult, op1=ALU.mult)
            dgs[n] = dg
        out_sb = out_pool.tile([P, D], FP16, name="out_sb", tag="ot")
        for a0, a1 in ((0, 512), (512, 768), (768, 1024)):
            w = a1 - a0
            accl = psum_pool.tile([P, 512], FP32, name="accl", tag="accl")
            for n in range(N):
                nc.tensor.matmul(accl[:, 0:w], dgs[n][:],
                                 pend["vts"][n][:, a0:a1],
                                 start=(n == 0), stop=(n == N - 1))
            if a0 == 0:
                nc.scalar.activation(out_sb[:, a0:a1], accl[:, 0:w], AF.Copy)
            else:
                nc.vector.tensor_copy(out_sb[:, a0:a1], accl[:, 0:w])
            stores.append((out_d[pend["b"], pend["t0"]:pend["t0"] + P, a0:a1],
                           out_sb[:, a0:a1]))

        for dst, src in stores:
            nc.sync.dma_start(dst, src)
    nc.compile()
    return nc


_NC = None


def _get_nc() -> bacc.Bacc:
    global _NC
    if _NC is None:
        _NC = _build_nc()
    return _NC


def _make_in_maps(V, w_l, norm_weight):
    V16 = np.asarray(V).astype(np.float16)
    w = np.asarray(w_l, np.float32) * np.asarray(norm_weight, np.float32)
    wbt = np.ascontiguousarray(np.broadcast_to(w.astype(np.float16), (P, D)))
    idm = np.eye(P, dtype=np.float16)
    in_maps = []
    for c in range(NCORES):
        vs = np.ascontiguousarray(V16[:, :, c * TSH:(c + 1) * TSH, :])
        in_maps.append({"v": vs, "wbt": wbt, "idm": idm})
    return in_maps


def _run(in_maps, trace=False, **kwargs):
    return run_bass_kernel_spmd(_get_nc(), in_maps, list(range(NCORES)),
                                trace=trace, **kwargs)


def kernel(V, w_l, norm_weight):
    res = _run(_make_in_maps(V, w_l, norm_weight))
    outs = [res.results[i]["out"] for i in range(NCORES)]
    return np.concatenate(outs, axis=1).astype(np.float32)
